# revision 3
# baseline (speedup 1.0000x reference)
"""Multi-head attention (B=64, S=577, D=768, H=12, Dh=64) on 8 TRN2 NeuronCores.

Strategy: data-parallel over batch (8 batches/core, no collectives).
Per core, for each batch:
  - X^T (host-pretransposed, bf16) -> SBUF [128, 6, 577]
  - Q^T, K^T = W^T-stationary matmuls -> [128, 6, 577] (head h lives at
    partitions (h%2)*64..+64 of d-tile h//2); bias and 1/sqrt(dh) folded in
    via DVE tensor_scalar on PSUM->SBUF copyback.
  - V = X^T-stationary matmul -> natural [t, o] layout [128, 5, 768] (+bias).
  - scores^T per head pair via PE row-tiling (two K=64 matmuls concurrent in
    rows 0-63 / 64-127), exp on ACT (PSUM->SBUF, bf16).
  - ctx^T + sums per head pair via PE col-tiling (two M=64 matmuls concurrent
    in cols 0-63 / 64-127); sums from an all-ones stationary.
  - softmax normalize = reciprocal_approx_fast(sums) * ctx (DVE), landing
    ctx^T in [128, 6, 577] bf16.
  - out = ctx^T-stationary matmul vs Wo^T + bias -> [577, 768] fp32 -> DRAM.
"""

import numpy as np
import ml_dtypes

import concourse.bacc as bacc
import concourse.tile as tile
import concourse.mybir as mybir
from concourse.bass_utils import run_bass_kernel_spmd

BF16 = mybir.dt.bfloat16
F32 = mybir.dt.float32

S = 577
D = 768
H = 12
DH = 64
NCORES = 8
B = 64
BPC = B // NCORES  # batches per core

# token tiles (used for k-tokens and t-tokens)
TT = [(i * 128, min(128, S - i * 128)) for i in range((S + 127) // 128)]  # 5 tiles
# q free-dim chunks (PSUM bank limit 512 fp32)
QC = [(0, 512), (512, S - 512)]
# output-feature chunks for V / out projections
OC = [(0, 512), (512, D - 512)]
NDT = D // 128  # 6 d-tiles


def build(bpc=BPC, name="attn_v1"):
    nc = bacc.Bacc("TRN2", target_bir_lowering=False, debug=False,
                   enable_partition_id=False, name=name)

    xt_d = nc.dram_tensor("xt", [bpc, D, S], BF16, kind="ExternalInput")
    wq_d = nc.dram_tensor("wqt", [D, D], BF16, kind="ExternalInput")
    wk_d = nc.dram_tensor("wkt", [D, D], BF16, kind="ExternalInput")
    wv_d = nc.dram_tensor("wvt", [D, D], BF16, kind="ExternalInput")
    wo_d = nc.dram_tensor("wot", [D, D], BF16, kind="ExternalInput")
    bq_d = nc.dram_tensor("bqp", [128, NDT], F32, kind="ExternalInput")
    bk_d = nc.dram_tensor("bkp", [128, NDT], F32, kind="ExternalInput")
    bv_d = nc.dram_tensor("bvb", [128, D], F32, kind="ExternalInput")
    bo_d = nc.dram_tensor("bob", [128, D], F32, kind="ExternalInput")
    out_d = nc.dram_tensor("out", [bpc, S, D], F32, kind="ExternalOutput")

    with tile.TileContext(nc) as tc:
        with (
            tc.tile_pool(name="singles", bufs=1) as singles,
            tc.tile_pool(name="xt", bufs=2) as xt_pool,
            tc.tile_pool(name="qk", bufs=2) as qk_pool,
            tc.tile_pool(name="v", bufs=2) as v_pool,
            tc.tile_pool(name="p", bufs=2) as p_pool,
            tc.tile_pool(name="ctx", bufs=2) as ctx_pool,
            tc.tile_pool(name="rec", bufs=2) as rec_pool,
            tc.tile_pool(name="osb", bufs=3) as o_pool,
            tc.tile_pool(name="ps_s", bufs=2, space="PSUM") as ps_s,
            tc.tile_pool(name="ps_mm", bufs=2, space="PSUM") as ps_mm,
            tc.tile_pool(name="ps_cs", bufs=2, space="PSUM") as ps_cs,
        ):
            # weights / biases resident in SBUF
            w_sb = {}
            for nm, d in (("q", wq_d), ("k", wk_d), ("v", wv_d), ("o", wo_d)):
                t = singles.tile([128, NDT, D], BF16, tag=f"w{nm}")
                nc.sync.dma_start(t[:], d.rearrange("(dt p) o -> p dt o", p=128))
                w_sb[nm] = t
            bq_sb = singles.tile([128, NDT], F32, tag="bq")
            nc.sync.dma_start(bq_sb[:], bq_d[:])
            bk_sb = singles.tile([128, NDT], F32, tag="bk")
            nc.sync.dma_start(bk_sb[:], bk_d[:])
            bv_sb = singles.tile([128, D], F32, tag="bv")
            nc.sync.dma_start(bv_sb[:], bv_d[:])
            bo_sb = singles.tile([128, D], F32, tag="bo")
            nc.sync.dma_start(bo_sb[:], bo_d[:])
            ones_sb = singles.tile([128, DH], BF16, tag="ones")
            nc.vector.memset(ones_sb[:], 1.0)

            for b in range(bpc):
                xt = xt_pool.tile([128, NDT, S], BF16)
                nc.sync.dma_start(xt[:], xt_d[b].rearrange("(dt p) s -> p dt s", p=128))

                # ---- Q^T / K^T projections: out[o_tile, q] ----
                qT = qk_pool.tile([128, NDT, S], BF16, tag="qT")
                kT = qk_pool.tile([128, NDT, S], BF16, tag="kT")
                for dst, w, bias, scale in ((qT, w_sb["q"], bq_sb, 0.125),
                                            (kT, w_sb["k"], bk_sb, None)):
                    for ot in range(NDT):
                        for qs, qw in QC:
                            ps = ps_mm.tile([128, 512], F32, tag="psmm")
                            for dt in range(NDT):
                                nc.tensor.matmul(
                                    ps[:, :qw],
                                    lhsT=w[:, dt, ot * 128:(ot + 1) * 128],
                                    rhs=xt[:, dt, qs:qs + qw],
                                    start=(dt == 0), stop=(dt == NDT - 1))
                            if scale is None:
                                nc.vector.tensor_scalar(
                                    dst[:, ot, qs:qs + qw], ps[:, :qw],
                                    bias[:, ot:ot + 1], None, mybir.AluOpType.add)
                            else:
                                nc.vector.tensor_scalar(
                                    dst[:, ot, qs:qs + qw], ps[:, :qw],
                                    bias[:, ot:ot + 1], scale,
                                    mybir.AluOpType.add, mybir.AluOpType.mult)

                # ---- V projection: natural [t, o] layout ----
                v_sb = v_pool.tile([128, len(TT), D], BF16)
                for ti, (ts_, tw) in enumerate(TT):
                    for os_, ow in OC:
                        ps = ps_mm.tile([128, 512], F32, tag="psmm")
                        for dt in range(NDT):
                            nc.tensor.matmul(
                                ps[:tw, :ow],
                                lhsT=xt[:, dt, ts_:ts_ + tw],
                                rhs=w_sb["v"][:, dt, os_:os_ + ow],
                                start=(dt == 0), stop=(dt == NDT - 1))
                        nc.vector.tensor_tensor(
                            v_sb[:tw, ti, os_:os_ + ow], ps[:tw, :ow],
                            bv_sb[:tw, os_:os_ + ow], mybir.AluOpType.add)

                # ---- attention, head pairs ----
                ctxT = ctx_pool.tile([128, NDT, S], BF16)
                for hp in range(H // 2):
                    pT = p_pool.tile([128, len(TT), 2, S], BF16)
                    for ki, (ks, kw) in enumerate(TT):
                        for qs, qw in QC:
                            sps = ps_s.tile([128, 2, 512], F32, tag="pss")
                            nc.tensor.matmul(
                                sps[:kw, 0, :qw],
                                lhsT=kT[0:64, hp, ks:ks + kw],
                                rhs=qT[0:64, hp, qs:qs + qw],
                                start=True, stop=True)
                            nc.tensor.matmul(
                                sps[:kw, 1, :qw],
                                lhsT=kT[64:128, hp, ks:ks + kw],
                                rhs=qT[64:128, hp, qs:qs + qw],
                                start=True, stop=True)
                            nc.scalar.activation(
                                pT[:kw, ki, :, qs:qs + qw], sps[:kw, :, :qw],
                                mybir.ActivationFunctionType.Exp)
                    for qs, qw in QC:
                        cps = ps_cs.tile([128, 512], F32, tag="pscs")
                        ssp = ps_cs.tile([128, 512], F32, tag="pscs")
                        for ki, (ks, kw) in enumerate(TT):
                            st, sp = (ki == 0), (ki == len(TT) - 1)
                            nc.tensor.matmul(
                                cps[0:64, :qw],
                                lhsT=v_sb[:kw, ki, (2 * hp) * DH:(2 * hp + 1) * DH],
                                rhs=pT[:kw, ki, 0, qs:qs + qw], start=st, stop=sp,
                                skip_group_check=True)
                            nc.tensor.matmul(
                                cps[64:128, :qw],
                                lhsT=v_sb[:kw, ki, (2 * hp + 1) * DH:(2 * hp + 2) * DH],
                                rhs=pT[:kw, ki, 1, qs:qs + qw], start=st, stop=sp,
                                skip_group_check=True)
                            nc.tensor.matmul(
                                ssp[0:64, :qw], lhsT=ones_sb[:kw, :],
                                rhs=pT[:kw, ki, 0, qs:qs + qw], start=st, stop=sp,
                                skip_group_check=True)
                            nc.tensor.matmul(
                                ssp[64:128, :qw], lhsT=ones_sb[:kw, :],
                                rhs=pT[:kw, ki, 1, qs:qs + qw], start=st, stop=sp,
                                skip_group_check=True)
                        rec = rec_pool.tile([128, 512], F32, tag="rec")
                        nc.vector.reciprocal_approx_fast(rec[:, :qw], ssp[:, :qw])
                        nc.vector.tensor_tensor(
                            ctxT[:, hp, qs:qs + qw], cps[:, :qw], rec[:, :qw],
                            mybir.AluOpType.mult)

                # ---- output projection ----
                for ti, (ts_, tw) in enumerate(TT):
                    osb = o_pool.tile([128, D], F32)
                    for os_, ow in OC:
                        ps = ps_mm.tile([128, 512], F32, tag="psmm")
                        for dt in range(NDT):
                            nc.tensor.matmul(
                                ps[:tw, :ow],
                                lhsT=ctxT[:, dt, ts_:ts_ + tw],
                                rhs=w_sb["o"][:, dt, os_:os_ + ow],
                                start=(dt == 0), stop=(dt == NDT - 1))
                        nc.vector.tensor_tensor(
                            osb[:tw, os_:os_ + ow], ps[:tw, :ow],
                            bo_sb[:tw, os_:os_ + ow], mybir.AluOpType.add)
                    nc.sync.dma_start(out_d[b, ts_:ts_ + tw, :], osb[:tw, :])

    nc.compile()
    return nc


_CACHE = {}


def _get_nc(bpc=BPC):
    if bpc not in _CACHE:
        _CACHE[bpc] = build(bpc)
    return _CACHE[bpc]


def prep_inputs(hidden_states, Wq, bq, Wk, bk, Wv, bv, Wo, bo, bpc=BPC):
    """Host-side preprocessing -> list of per-core input dicts."""
    hs = np.asarray(hidden_states, dtype=np.float32)
    xt = np.ascontiguousarray(hs.transpose(0, 2, 1)).astype(ml_dtypes.bfloat16)
    shared = {
        "wqt": np.ascontiguousarray(np.asarray(Wq, np.float32).T).astype(ml_dtypes.bfloat16),
        "wkt": np.ascontiguousarray(np.asarray(Wk, np.float32).T).astype(ml_dtypes.bfloat16),
        "wvt": np.ascontiguousarray(np.asarray(Wv, np.float32).T).astype(ml_dtypes.bfloat16),
        "wot": np.ascontiguousarray(np.asarray(Wo, np.float32).T).astype(ml_dtypes.bfloat16),
        "bqp": np.ascontiguousarray(np.asarray(bq, np.float32).reshape(NDT, 128).T),
        "bkp": np.ascontiguousarray(np.asarray(bk, np.float32).reshape(NDT, 128).T),
        "bvb": np.ascontiguousarray(np.broadcast_to(np.asarray(bv, np.float32), (128, D))),
        "bob": np.ascontiguousarray(np.broadcast_to(np.asarray(bo, np.float32), (128, D))),
    }
    n_cores = xt.shape[0] // bpc
    return [{"xt": np.ascontiguousarray(xt[c * bpc:(c + 1) * bpc]), **shared}
            for c in range(n_cores)]


def kernel(hidden_states, Wq, bq, Wk, bk, Wv, bv, Wo, bo):
    nc = _get_nc(BPC)
    in_maps = prep_inputs(hidden_states, Wq, bq, Wk, bk, Wv, bv, Wo, bo, BPC)
    res = run_bass_kernel_spmd(nc, in_maps, core_ids=list(range(NCORES)))
    out = np.concatenate([res.results[c]["out"] for c in range(NCORES)], axis=0)
    return out.astype(np.float32)
